# revision 43
# baseline (speedup 1.0000x reference)
"""NCN link predictor (nn_NCNPredictor_77292231459355) on 8 Trainium2 cores.

Strategy (B-sharded per the sharding hint): the 1024 target pairs are split
128 per core (one SBUF partition per pair). The host symmetrizes edge_index
and builds padded CSR-style adjacency rows for each pair's i-side and j-side
(ni, nj). Instead of shipping the full 51MB x to every core, the host ships
only the CSR slice each core can possibly touch: j-side slots are pruned by
membership in their 32-pair group's i-side node union (slots outside it are
provably weight-0; false positives remain and are counted exactly on
device), and the x rows of the surviving slots are gathered host-side
(bf16). Mirror pairs (tar_j[b] == tar_i[b']) are placed in different groups
so one pair's full j-list never survives the filter. On device, each core:
  1. computes c[b,q] = multiplicity of j-slot q in i's full row via a
     broadcast equality + grouped reduce (the A_i*A_j intersection),
  2. computes xcn[b,:] = sum_q c[b,q] * x[nj[b,q],:] as a dense weighted
     reduction over the gathered rows (no indirect DMA, no top-k),
  3. computes xij = x[tar_i]*x[tar_j] and the 2-layer MLP head on
     TensorE/ACT (biases/W2 replicated via rank-1 ones-outer-products).
All per-pair inputs ship as ONE bf16 tensor per core (f32 fields bitcast),
in two DMAs so the equality pass starts as soon as the id lists land.
Host un-permutes the 8 per-core [128] score slices into the final [1024].
"""

import numpy as np
import ml_dtypes

N_NODES = 100000
B = 1024
D = 128
DH = 512
N_CORES = 8
BL = B // N_CORES  # 128 pairs per core = SBUF partition dim
GRP = 32           # filter-group size (pairs) for the CSR membership prune

_compiled_cache: dict = {}
_pcut: dict = {}  # (si, sj) -> count of w2 >= 0 after pos-first permutation


def _padded_rows(src, dst, targets, sentinel):
    """Padded adjacency rows (with multiplicity as repeated entries) of the
    symmetric edge list at `targets` -> float32 [B, S] (S = max degree,
    padded to a multiple of 8, >= 8). Pad slots hold `sentinel`."""
    b = targets.shape[0]
    pos = np.full(N_NODES, -1, np.int32)
    pos[targets] = np.arange(b, dtype=np.int32)
    r = pos[src]
    m = r >= 0
    rows = r[m].astype(np.int64)
    cols = dst[m].astype(np.int64)
    order = np.argsort(rows, kind="stable")
    rows = rows[order]
    cols = cols[order]
    cnt = np.bincount(rows, minlength=b)
    s = max(8, (int(cnt.max()) + 7) // 8 * 8)
    starts = np.zeros(b + 1, np.int64)
    np.cumsum(cnt, out=starts[1:])
    within = np.arange(rows.size, dtype=np.int64) - starts[rows]
    out = np.full((b, s), sentinel, np.float32)
    out[rows, within] = cols.astype(np.float32)
    return out


def _big_layout(si, sj):
    """bf16-column offsets within the merged per-core input block. The f32
    fields (ni, nj, b2) sit at even offsets so they can be bitcast."""
    lay = {}
    off = 0
    for name, w in [
        ("ni", 2 * si), ("nj", 2 * sj), ("b2", 2),
        ("xiT", BL), ("xjT", BL), ("w1a", DH), ("w1b", DH), ("xnj", D * sj),
    ]:
        lay[name] = (off, w)
        off += w
    listsw = 2 * si + 2 * sj + 2
    return lay, listsw, off


def _build_bass(si, sj, repeat=1):
    """repeat>1 unrolls the whole body N times over the same tiles (serial
    via WAW deps) — used only for amplified wall-clock timing."""
    import concourse.bass as bass
    import concourse.tile as tile
    from concourse import bacc, mybir

    f32 = mybir.dt.float32
    bf16 = mybir.dt.bfloat16
    i32 = mybir.dt.int32
    i16 = mybir.dt.int16

    lay, listsw, bigw = _big_layout(si, sj)

    nc = bacc.Bacc(
        "TRN2", target_bir_lowering=False, debug=False, num_devices=N_CORES
    )

    # int16 container: id lists / b2 are f32 and features are bf16, all
    # bitcast below — int16 keeps the sim's DMA finiteness check happy.
    pcut = _pcut[(si, sj)]
    assert 0 < pcut < DH, pcut

    big_d = nc.dram_tensor("big", [BL, bigw], i16, kind="ExternalInput").ap()
    sml_d = nc.dram_tensor("sml", [1, DH + BL], bf16, kind="ExternalInput").ap()
    out_d = nc.dram_tensor("out", [BL, 1], f32, kind="ExternalOutput").ap()

    with tile.TileContext(nc) as tc:
        with (
            tc.tile_pool(name="sb", bufs=2) as sb,
            tc.tile_pool(name="ps", bufs=2, space="PSUM") as ps,
        ):
          for _rep in range(repeat):
            big = sb.tile([BL, bigw], i16, tag="big")
            # split by consumer so the chunks ride parallel DMA queues and
            # land in the order the compute needs them
            xnj_off = lay["xnj"][0]
            nc.sync.dma_start(big[:, 0:listsw], big_d[:, 0:listsw])
            nc.sync.dma_start(big[:, xnj_off:bigw], big_d[:, xnj_off:bigw])
            nc.sync.dma_start(big[:, listsw:xnj_off], big_d[:, listsw:xnj_off])
            sml = sb.tile([1, DH + BL], bf16, tag="sml")
            nc.sync.dma_start(sml[:], sml_d[:])

            def bslice(name):
                off, w = lay[name]
                return big[:, off : off + w]

            nif = bslice("ni").bitcast(f32)
            njf = bslice("nj").bitcast(f32)
            b2b = bslice("b2").bitcast(f32)
            xiT = bslice("xiT").bitcast(bf16)
            xjT = bslice("xjT").bitcast(bf16)
            w1a = bslice("w1a").bitcast(bf16)
            w1b = bslice("w1b").bitcast(bf16)
            xnj = bslice("xnj").bitcast(bf16)

            b1r = sml[0:1, 0:DH]
            ones1 = sml[0:1, DH : DH + BL]

            # --- intersection counts: c[b,q] = sum_p (NJ[b,q] == NI[b,p]) ---
            eq3 = sb.tile([BL, sj * si], bf16, tag="eq3")
            nc.vector.tensor_tensor(
                out=eq3[:].rearrange("p (q i) -> p q i", i=si),
                in0=njf.unsqueeze(2).broadcast_to([BL, sj, si]),
                in1=nif.unsqueeze(1).broadcast_to([BL, sj, si]),
                op=mybir.AluOpType.is_equal,
            )
            cmat = sb.tile([BL, sj], bf16, tag="cmat")
            with nc.allow_low_precision(
                reason="counts <= 96 are exact integers in bf16"
            ):
                nc.vector.tensor_reduce(
                    out=cmat[:],
                    in_=eq3[:].rearrange("p (q i) -> p q i", i=si),
                    axis=mybir.AxisListType.X,
                    op=mybir.AluOpType.add,
                )

            # --- xcn[b,d] = sum_q c[b,q] * xnj[b,d,q] (dense weighted sum) ---
            xcn = sb.tile([BL, D], f32, tag="xcn")
            prod = sb.tile([BL, D * sj], bf16, tag="prod")
            nc.vector.tensor_tensor(
                out=prod[:].rearrange("p (d q) -> p d q", q=sj),
                in0=xnj.rearrange("p (d q) -> p d q", q=sj),
                in1=cmat[:].unsqueeze(1).broadcast_to([BL, D, sj]),
                op=mybir.AluOpType.mult,
            )
            nc.vector.tensor_reduce(
                out=xcn[:],
                in_=prod[:].rearrange("p (d q) -> p d q", q=sj),
                axis=mybir.AxisListType.X,
                op=mybir.AluOpType.add,
            )
            # xijT = (x[tar_i] * x[tar_j])^T — born transposed, ready as lhsT
            xst0 = sb.tile([BL, BL], bf16, tag="xst0")
            nc.vector.tensor_mul(out=xst0[:], in0=xiT, in1=xjT)

            # transpose identity, built after the front so it doesn't stall
            # the DVE queue ahead of the intersection math
            iot = sb.tile([BL, BL], i32, tag="iot")
            nc.gpsimd.iota(
                out=iot[:], pattern=[[-1, BL]], base=0, channel_multiplier=1
            )
            ident = sb.tile([BL, BL], f32, tag="ident")
            nc.vector.tensor_single_scalar(
                out=ident[:], in_=iot[:], scalar=0, op=mybir.AluOpType.is_equal
            )

            # --- MLP head: out = relu(xs @ W1 + b1) @ W2 + b2 ---
            pst1 = ps.tile([BL, BL], f32, tag="pst1")
            nc.tensor.transpose(out=pst1[:], in_=xcn[:], identity=ident[:])
            xst1 = sb.tile([BL, BL], bf16, tag="xst1")
            nc.scalar.copy(out=xst1[:], in_=pst1[:])

            psh = ps.tile([BL, DH], f32, tag="psh")
            nc.tensor.matmul(
                psh[:], lhsT=ones1, rhs=b1r,
                start=True, stop=False, skip_group_check=True,
            )
            nc.tensor.matmul(
                psh[:], lhsT=xst0[:], rhs=w1a,
                start=False, stop=False, skip_group_check=True,
            )
            nc.tensor.matmul(
                psh[:], lhsT=xst1[:], rhs=w1b,
                start=False, stop=True, skip_group_check=True,
            )
            # W2 is folded into W1 host-side (psh holds z*w2, positives-first
            # hidden permutation): h@W2 = sum relu(z'[0:P]) - sum relu(-z'[P:]).
            # ACT's accum_out sums the free dim of each branch.
            dumpa = sb.tile([BL, pcut], bf16, tag="dumpa")
            acca = sb.tile([BL, 1], f32, tag="acca")
            nc.scalar.activation(
                out=dumpa[:], in_=psh[:, 0:pcut],
                func=mybir.ActivationFunctionType.Relu, accum_out=acca[:],
            )
            dumpb = sb.tile([BL, DH - pcut], bf16, tag="dumpb")
            accb = sb.tile([BL, 1], f32, tag="accb")
            nc.scalar.activation(
                out=dumpb[:], in_=psh[:, pcut:DH],
                func=mybir.ActivationFunctionType.Relu, scale=-1.0,
                accum_out=accb[:],
            )
            res0 = sb.tile([BL, 1], f32, tag="res0")
            nc.vector.scalar_tensor_tensor(
                out=res0[:], in0=accb[:], scalar=-1.0, in1=acca[:],
                op0=mybir.AluOpType.mult, op1=mybir.AluOpType.add,
            )
            res = sb.tile([BL, 1], f32, tag="res")
            nc.scalar.activation(
                out=res[:], in_=res0[:],
                func=mybir.ActivationFunctionType.Identity, bias=b2b,
            )
            nc.sync.dma_start(out_d[:], res[:])

    nc.compile()
    return nc


def _prepare(x, edge_index, tar_ei, W1, b1, W2, b2):
    e0 = np.asarray(edge_index[0]).astype(np.int64)
    e1 = np.asarray(edge_index[1]).astype(np.int64)
    src = np.concatenate([e0, e1])
    dst = np.concatenate([e1, e0])
    tar_i = np.asarray(tar_ei[0]).astype(np.int64)
    tar_j = np.asarray(tar_ei[1]).astype(np.int64)

    ni = _padded_rows(src, dst, tar_i, sentinel=-1.0)
    nj = _padded_rows(src, dst, tar_j, sentinel=-2.0)
    si = ni.shape[1]

    # Pair->group assignment: if tar_j[b] == tar_i[b'] for a same-group b',
    # pair b's whole j-list survives the membership filter below (mirror
    # adjacency), inflating the padded width for everyone. Since tar_i
    # entries are unique, each b has at most one such b': place them in
    # different groups. asg[k] = original pair handled by slot k.
    asg = np.arange(B)
    pos_i = {int(t): b for b, t in enumerate(tar_i)}
    slot_of = np.arange(B)
    for _sweep in range(4):  # a swap can re-collide an earlier pair; re-scan
        dirty = False
        for b in range(B):
            bp = pos_i.get(int(tar_j[b]), None)
            if bp is None or bp == b:
                continue
            if slot_of[b] // GRP == slot_of[bp] // GRP:
                for p in range(B):  # swap b with a partner from another group
                    if slot_of[p] // GRP == slot_of[b] // GRP:
                        continue
                    pc = pos_i.get(int(tar_j[p]), None)
                    if pc is not None and slot_of[pc] // GRP == slot_of[b] // GRP:
                        continue
                    sb, sp = slot_of[b], slot_of[p]
                    asg[sb], asg[sp] = p, b
                    slot_of[b], slot_of[p] = sp, sb
                    dirty = True
                    break
        if not dirty:
            break
    ni = ni[asg]
    nj = nj[asg]
    tar_i = tar_i[asg]
    tar_j = tar_j[asg]

    # Grouped CSR pruning: a j-slot can only intersect its pair's i-list,
    # which is a subset of its 32-pair group's i-side node union — so slots
    # outside that union are provably weight-0 and need not ship. (False
    # positives from other pairs' i-lists still ship and get exact c counts
    # on device; per-slot multiplicity is preserved. Result is bit-exact.)
    keep = np.zeros(nj.shape, bool)
    for g in range(B // GRP):
        sl = slice(g * GRP, (g + 1) * GRP)
        u = np.unique(ni[sl])
        u = u[u >= 0.0]
        keep[sl] = np.isin(nj[sl], u)
    cnt = keep.sum(axis=1)
    sj = max(2, (int(cnt.max()) + 1) // 2 * 2)
    njk = np.full((B, sj), -2.0, np.float32)
    rows, qs = np.nonzero(keep)
    within = np.arange(rows.size) - np.searchsorted(rows, rows)
    njk[rows, within] = nj[rows, qs]
    nj = njk

    # Symmetric prune of the i-side: c[b,q] only queries the surviving nj
    # values, so i-slots whose value is outside the group's surviving-nj
    # union can never match. Value-based, so multiplicity of every queried
    # value is preserved -> counts stay exact.
    keep_i = np.zeros(ni.shape, bool)
    for g in range(B // GRP):
        sl = slice(g * GRP, (g + 1) * GRP)
        u = np.unique(nj[sl])
        u = u[u >= 0.0]
        keep_i[sl] = np.isin(ni[sl], u)
    cnt_i = keep_i.sum(axis=1)
    si = max(2, (int(cnt_i.max()) + 1) // 2 * 2)
    nik = np.full((B, si), -1.0, np.float32)
    rows, ps_ = np.nonzero(keep_i)
    within = np.arange(rows.size) - np.searchsorted(rows, rows)
    nik[rows, within] = ni[rows, ps_]
    ni = nik
    assert si <= 127 and sj <= 512, (si, sj)

    x = np.asarray(x, dtype=np.float32)
    w1 = np.asarray(W1, dtype=np.float32)
    bf = ml_dtypes.bfloat16

    # gather the x rows of every surviving j-slot: [B, sj, D] -> [B, D, sj]
    nj_ids = np.where(nj >= 0.0, nj, 0.0).astype(np.int64)
    xnj = x[nj_ids.reshape(-1)].reshape(B, sj, D).transpose(0, 2, 1)
    xnj = np.ascontiguousarray(xnj.reshape(B, D * sj)).astype(bf)

    b2col = np.full((B, 1), np.float32(np.asarray(b2).reshape(-1)[0]), np.float32)

    # Fold W2 into W1 (relu(z)@w2 == sum relu(z*w2[pos]) - sum relu(-z*w2[neg]))
    # with a positives-first hidden permutation so each sign is contiguous.
    w2v = np.asarray(W2, np.float32).reshape(DH)
    b1v = np.asarray(b1, np.float32).reshape(DH)
    order = np.argsort(w2v < 0, kind="stable")
    pcut = int((w2v >= 0).sum())
    _pcut[(si, sj)] = pcut
    w1f = (w1 * w2v[None, :])[:, order]
    b1f = (b1v * w2v)[order]
    w1cat = np.concatenate([w1f[0:D], w1f[D : 2 * D]], axis=1).astype(bf)
    # per-core transposed xi/xj blocks: row (core, d) holds x[tar_*[core, b], d]
    xiT = np.empty((B, BL), bf)
    xjT = np.empty((B, BL), bf)
    for ci in range(N_CORES):
        sl = slice(ci * BL, (ci + 1) * BL)
        xiT[sl] = np.ascontiguousarray(x[tar_i[sl]].astype(bf).T)
        xjT[sl] = np.ascontiguousarray(x[tar_j[sl]].astype(bf).T)
    big = np.concatenate(
        [
            np.ascontiguousarray(ni).view(np.int16),
            np.ascontiguousarray(nj).view(np.int16),
            b2col.view(np.int16),
            xiT.view(np.int16),
            xjT.view(np.int16),
            np.tile(w1cat, (N_CORES, 1)).view(np.int16),
            xnj.view(np.int16),
        ],
        axis=1,
    )
    lay, listsw, bigw = _big_layout(si, sj)
    assert big.shape == (B, bigw), (big.shape, bigw)

    sml = np.concatenate([b1f, np.ones(BL, np.float32)]).reshape(1, DH + BL).astype(bf)

    in_maps = []
    for ci in range(N_CORES):
        sl = slice(ci * BL, (ci + 1) * BL)
        in_maps.append({
            "big": np.ascontiguousarray(big[sl]),
            "sml": sml,
        })
    return in_maps, si, sj, asg


def kernel(x, edge_index, tar_ei, W1, b1, W2, b2):
    from concourse.bass_utils import run_bass_kernel_spmd

    in_maps, si, sj, asg = _prepare(x, edge_index, tar_ei, W1, b1, W2, b2)

    key = (si, sj)
    if key not in _compiled_cache:
        _compiled_cache[key] = _build_bass(si, sj)
    nc = _compiled_cache[key]

    res = run_bass_kernel_spmd(nc, in_maps, list(range(N_CORES)))
    slots = np.concatenate(
        [res.results[ci]["out"].reshape(BL) for ci in range(N_CORES)]
    ).astype(np.float32)
    out = np.empty(B, np.float32)
    out[asg] = slots
    return out


# revision 44
# speedup vs baseline: 2.5597x; 2.5597x over previous
"""NCN link predictor (nn_NCNPredictor_77292231459355) on 8 Trainium2 cores.

Strategy (B-sharded per the sharding hint): the 1024 target pairs are split
128 per core (one SBUF partition per pair). The host symmetrizes edge_index
and builds padded CSR-style adjacency rows for each pair's i-side and j-side
(ni, nj). Instead of shipping the full 51MB x to every core, the host ships
only the CSR slice each core can possibly touch: j-side slots are pruned by
membership in their 32-pair group's i-side node union (slots outside it are
provably weight-0; false positives remain and are counted exactly on
device), and the x rows of the surviving slots are gathered host-side
(bf16). Mirror pairs (tar_j[b] == tar_i[b']) are placed in different groups
so one pair's full j-list never survives the filter. On device, each core:
  1. computes c[b,q] = multiplicity of j-slot q in i's full row via a
     broadcast equality + grouped reduce (the A_i*A_j intersection),
  2. computes xcn[b,:] = sum_q c[b,q] * x[nj[b,q],:] as a dense weighted
     reduction over the gathered rows (no indirect DMA, no top-k),
  3. computes xij = x[tar_i]*x[tar_j] and the 2-layer MLP head on
     TensorE/ACT (biases/W2 replicated via rank-1 ones-outer-products).
All per-pair inputs ship as ONE bf16 tensor per core (f32 fields bitcast),
in two DMAs so the equality pass starts as soon as the id lists land.
Host un-permutes the 8 per-core [128] score slices into the final [1024].
"""

import numpy as np
import ml_dtypes

N_NODES = 100000
B = 1024
D = 128
DH = 512
N_CORES = 8
BL = B // N_CORES  # 128 pairs per core = SBUF partition dim
GRP = 32           # filter-group size (pairs) for the CSR membership prune

_compiled_cache: dict = {}
_pcut: dict = {}  # (si, sj) -> count of w2 >= 0 after pos-first permutation


def _padded_rows(src, dst, targets, sentinel):
    """Padded adjacency rows (with multiplicity as repeated entries) of the
    symmetric edge list at `targets` -> float32 [B, S] (S = max degree,
    padded to a multiple of 8, >= 8). Pad slots hold `sentinel`."""
    b = targets.shape[0]
    pos = np.full(N_NODES, -1, np.int32)
    pos[targets] = np.arange(b, dtype=np.int32)
    r = pos[src]
    m = r >= 0
    rows = r[m].astype(np.int64)
    cols = dst[m].astype(np.int64)
    order = np.argsort(rows, kind="stable")
    rows = rows[order]
    cols = cols[order]
    cnt = np.bincount(rows, minlength=b)
    s = max(8, (int(cnt.max()) + 7) // 8 * 8)
    starts = np.zeros(b + 1, np.int64)
    np.cumsum(cnt, out=starts[1:])
    within = np.arange(rows.size, dtype=np.int64) - starts[rows]
    out = np.full((b, s), sentinel, np.float32)
    out[rows, within] = cols.astype(np.float32)
    return out


def _big_layout(si, sj):
    """bf16-column offsets within the merged per-core input block. The f32
    fields (ni, nj, b2) sit at even offsets so they can be bitcast."""
    lay = {}
    off = 0
    for name, w in [
        ("ni", 2 * si), ("nj", 2 * sj), ("b2", 2),
        ("xiT", BL), ("xjT", BL), ("w1a", DH), ("w1b", DH), ("xnj", D * sj),
    ]:
        lay[name] = (off, w)
        off += w
    listsw = 2 * si + 2 * sj + 2
    return lay, listsw, off


def _build_bass(si, sj, repeat=1):
    """repeat>1 unrolls the whole body N times over the same tiles (serial
    via WAW deps) — used only for amplified wall-clock timing."""
    import concourse.bass as bass
    import concourse.tile as tile
    from concourse import bacc, mybir

    f32 = mybir.dt.float32
    bf16 = mybir.dt.bfloat16
    i32 = mybir.dt.int32
    i16 = mybir.dt.int16

    lay, listsw, bigw = _big_layout(si, sj)

    nc = bacc.Bacc(
        "TRN2", target_bir_lowering=False, debug=False, num_devices=N_CORES
    )

    # int16 container: id lists / b2 are f32 and features are bf16, all
    # bitcast below — int16 keeps the sim's DMA finiteness check happy.
    pcut = _pcut[(si, sj)]
    assert 0 < pcut < DH, pcut

    big_d = nc.dram_tensor("big", [BL, bigw], i16, kind="ExternalInput").ap()
    sml_d = nc.dram_tensor("sml", [1, DH + BL], bf16, kind="ExternalInput").ap()
    out_d = nc.dram_tensor("out", [BL, 1], f32, kind="ExternalOutput").ap()

    with tile.TileContext(nc) as tc:
        with (
            tc.tile_pool(name="sb", bufs=2) as sb,
            tc.tile_pool(name="ps", bufs=2, space="PSUM") as ps,
        ):
          for _rep in range(repeat):
            big = sb.tile([BL, bigw], i16, tag="big")
            # split by consumer so the chunks ride parallel DMA queues and
            # land in the order the compute needs them
            xnj_off = lay["xnj"][0]
            nc.sync.dma_start(big[:, 0:listsw], big_d[:, 0:listsw])
            nc.sync.dma_start(big[:, xnj_off:bigw], big_d[:, xnj_off:bigw])
            nc.sync.dma_start(big[:, listsw:xnj_off], big_d[:, listsw:xnj_off])
            sml = sb.tile([1, DH + BL], bf16, tag="sml")
            nc.sync.dma_start(sml[:], sml_d[:])

            def bslice(name):
                off, w = lay[name]
                return big[:, off : off + w]

            nif = bslice("ni").bitcast(f32)
            njf = bslice("nj").bitcast(f32)
            b2b = bslice("b2").bitcast(f32)
            xiT = bslice("xiT").bitcast(bf16)
            xjT = bslice("xjT").bitcast(bf16)
            w1a = bslice("w1a").bitcast(bf16)
            w1b = bslice("w1b").bitcast(bf16)
            xnj = bslice("xnj").bitcast(bf16)

            b1r = sml[0:1, 0:DH]
            ones1 = sml[0:1, DH : DH + BL]

            # --- intersection counts: c[b,q] = sum_p (NJ[b,q] == NI[b,p]) ---
            eq3 = sb.tile([BL, sj * si], bf16, tag="eq3")
            nc.vector.tensor_tensor(
                out=eq3[:].rearrange("p (q i) -> p q i", i=si),
                in0=njf.unsqueeze(2).broadcast_to([BL, sj, si]),
                in1=nif.unsqueeze(1).broadcast_to([BL, sj, si]),
                op=mybir.AluOpType.is_equal,
            )
            cmat = sb.tile([BL, sj], bf16, tag="cmat")
            with nc.allow_low_precision(
                reason="counts <= 96 are exact integers in bf16"
            ):
                nc.vector.tensor_reduce(
                    out=cmat[:],
                    in_=eq3[:].rearrange("p (q i) -> p q i", i=si),
                    axis=mybir.AxisListType.X,
                    op=mybir.AluOpType.add,
                )

            # --- xcn[b,d] = sum_q c[b,q] * xnj[b,d,q] (dense weighted sum) ---
            xcn = sb.tile([BL, D], f32, tag="xcn")
            prod = sb.tile([BL, D * sj], bf16, tag="prod")
            nc.vector.tensor_tensor(
                out=prod[:].rearrange("p (d q) -> p d q", q=sj),
                in0=xnj.rearrange("p (d q) -> p d q", q=sj),
                in1=cmat[:].unsqueeze(1).broadcast_to([BL, D, sj]),
                op=mybir.AluOpType.mult,
            )
            nc.vector.tensor_reduce(
                out=xcn[:],
                in_=prod[:].rearrange("p (d q) -> p d q", q=sj),
                axis=mybir.AxisListType.X,
                op=mybir.AluOpType.add,
            )
            # xijT = (x[tar_i] * x[tar_j])^T — born transposed, ready as lhsT
            xst0 = sb.tile([BL, BL], bf16, tag="xst0")
            nc.vector.tensor_mul(out=xst0[:], in0=xiT, in1=xjT)

            # transpose identity, built after the front so it doesn't stall
            # the DVE queue ahead of the intersection math
            iot = sb.tile([BL, BL], i32, tag="iot")
            nc.gpsimd.iota(
                out=iot[:], pattern=[[-1, BL]], base=0, channel_multiplier=1
            )
            ident = sb.tile([BL, BL], f32, tag="ident")
            nc.gpsimd.tensor_single_scalar(
                out=ident[:], in_=iot[:], scalar=0, op=mybir.AluOpType.is_equal
            )

            # --- MLP head: out = relu(xs @ W1 + b1) @ W2 + b2 ---
            pst1 = ps.tile([BL, BL], f32, tag="pst1")
            nc.tensor.transpose(out=pst1[:], in_=xcn[:], identity=ident[:])
            xst1 = sb.tile([BL, BL], bf16, tag="xst1")
            nc.scalar.copy(out=xst1[:], in_=pst1[:])

            psh = ps.tile([BL, DH], f32, tag="psh")
            nc.tensor.matmul(
                psh[:], lhsT=ones1, rhs=b1r,
                start=True, stop=False, skip_group_check=True,
            )
            nc.tensor.matmul(
                psh[:], lhsT=xst0[:], rhs=w1a,
                start=False, stop=False, skip_group_check=True,
            )
            nc.tensor.matmul(
                psh[:], lhsT=xst1[:], rhs=w1b,
                start=False, stop=True, skip_group_check=True,
            )
            # W2 is folded into W1 host-side (psh holds z*w2, positives-first
            # hidden permutation): h@W2 = sum relu(z'[0:P]) - sum relu(-z'[P:]).
            # ACT's accum_out sums the free dim of each branch.
            dumpa = sb.tile([BL, pcut], bf16, tag="dumpa")
            acca = sb.tile([BL, 1], f32, tag="acca")
            nc.scalar.activation(
                out=dumpa[:], in_=psh[:, 0:pcut],
                func=mybir.ActivationFunctionType.Relu, accum_out=acca[:],
            )
            dumpb = sb.tile([BL, DH - pcut], bf16, tag="dumpb")
            accb = sb.tile([BL, 1], f32, tag="accb")
            nc.scalar.activation(
                out=dumpb[:], in_=psh[:, pcut:DH],
                func=mybir.ActivationFunctionType.Relu, scale=-1.0,
                accum_out=accb[:],
            )
            res0 = sb.tile([BL, 1], f32, tag="res0")
            nc.vector.scalar_tensor_tensor(
                out=res0[:], in0=accb[:], scalar=-1.0, in1=acca[:],
                op0=mybir.AluOpType.mult, op1=mybir.AluOpType.add,
            )
            res = sb.tile([BL, 1], f32, tag="res")
            nc.scalar.activation(
                out=res[:], in_=res0[:],
                func=mybir.ActivationFunctionType.Identity, bias=b2b,
            )
            nc.sync.dma_start(out_d[:], res[:])

    nc.compile()
    return nc


def _prepare(x, edge_index, tar_ei, W1, b1, W2, b2):
    e0 = np.asarray(edge_index[0]).astype(np.int64)
    e1 = np.asarray(edge_index[1]).astype(np.int64)
    src = np.concatenate([e0, e1])
    dst = np.concatenate([e1, e0])
    tar_i = np.asarray(tar_ei[0]).astype(np.int64)
    tar_j = np.asarray(tar_ei[1]).astype(np.int64)

    ni = _padded_rows(src, dst, tar_i, sentinel=-1.0)
    nj = _padded_rows(src, dst, tar_j, sentinel=-2.0)
    si = ni.shape[1]

    # Pair->group assignment: if tar_j[b] == tar_i[b'] for a same-group b',
    # pair b's whole j-list survives the membership filter below (mirror
    # adjacency), inflating the padded width for everyone. Since tar_i
    # entries are unique, each b has at most one such b': place them in
    # different groups. asg[k] = original pair handled by slot k.
    asg = np.arange(B)
    pos_i = {int(t): b for b, t in enumerate(tar_i)}
    slot_of = np.arange(B)
    for _sweep in range(4):  # a swap can re-collide an earlier pair; re-scan
        dirty = False
        for b in range(B):
            bp = pos_i.get(int(tar_j[b]), None)
            if bp is None or bp == b:
                continue
            if slot_of[b] // GRP == slot_of[bp] // GRP:
                for p in range(B):  # swap b with a partner from another group
                    if slot_of[p] // GRP == slot_of[b] // GRP:
                        continue
                    pc = pos_i.get(int(tar_j[p]), None)
                    if pc is not None and slot_of[pc] // GRP == slot_of[b] // GRP:
                        continue
                    sb, sp = slot_of[b], slot_of[p]
                    asg[sb], asg[sp] = p, b
                    slot_of[b], slot_of[p] = sp, sb
                    dirty = True
                    break
        if not dirty:
            break
    ni = ni[asg]
    nj = nj[asg]
    tar_i = tar_i[asg]
    tar_j = tar_j[asg]

    # Grouped CSR pruning: a j-slot can only intersect its pair's i-list,
    # which is a subset of its 32-pair group's i-side node union — so slots
    # outside that union are provably weight-0 and need not ship. (False
    # positives from other pairs' i-lists still ship and get exact c counts
    # on device; per-slot multiplicity is preserved. Result is bit-exact.)
    keep = np.zeros(nj.shape, bool)
    for g in range(B // GRP):
        sl = slice(g * GRP, (g + 1) * GRP)
        u = np.unique(ni[sl])
        u = u[u >= 0.0]
        keep[sl] = np.isin(nj[sl], u)
    cnt = keep.sum(axis=1)
    sj = max(2, (int(cnt.max()) + 1) // 2 * 2)
    njk = np.full((B, sj), -2.0, np.float32)
    rows, qs = np.nonzero(keep)
    within = np.arange(rows.size) - np.searchsorted(rows, rows)
    njk[rows, within] = nj[rows, qs]
    nj = njk

    # Symmetric prune of the i-side: c[b,q] only queries the surviving nj
    # values, so i-slots whose value is outside the group's surviving-nj
    # union can never match. Value-based, so multiplicity of every queried
    # value is preserved -> counts stay exact.
    keep_i = np.zeros(ni.shape, bool)
    for g in range(B // GRP):
        sl = slice(g * GRP, (g + 1) * GRP)
        u = np.unique(nj[sl])
        u = u[u >= 0.0]
        keep_i[sl] = np.isin(ni[sl], u)
    cnt_i = keep_i.sum(axis=1)
    si = max(2, (int(cnt_i.max()) + 1) // 2 * 2)
    nik = np.full((B, si), -1.0, np.float32)
    rows, ps_ = np.nonzero(keep_i)
    within = np.arange(rows.size) - np.searchsorted(rows, rows)
    nik[rows, within] = ni[rows, ps_]
    ni = nik
    assert si <= 127 and sj <= 512, (si, sj)

    x = np.asarray(x, dtype=np.float32)
    w1 = np.asarray(W1, dtype=np.float32)
    bf = ml_dtypes.bfloat16

    # gather the x rows of every surviving j-slot: [B, sj, D] -> [B, D, sj]
    nj_ids = np.where(nj >= 0.0, nj, 0.0).astype(np.int64)
    xnj = x[nj_ids.reshape(-1)].reshape(B, sj, D).transpose(0, 2, 1)
    xnj = np.ascontiguousarray(xnj.reshape(B, D * sj)).astype(bf)

    b2col = np.full((B, 1), np.float32(np.asarray(b2).reshape(-1)[0]), np.float32)

    # Fold W2 into W1 (relu(z)@w2 == sum relu(z*w2[pos]) - sum relu(-z*w2[neg]))
    # with a positives-first hidden permutation so each sign is contiguous.
    w2v = np.asarray(W2, np.float32).reshape(DH)
    b1v = np.asarray(b1, np.float32).reshape(DH)
    order = np.argsort(w2v < 0, kind="stable")
    pcut = int((w2v >= 0).sum())
    _pcut[(si, sj)] = pcut
    w1f = (w1 * w2v[None, :])[:, order]
    b1f = (b1v * w2v)[order]
    w1cat = np.concatenate([w1f[0:D], w1f[D : 2 * D]], axis=1).astype(bf)
    # per-core transposed xi/xj blocks: row (core, d) holds x[tar_*[core, b], d]
    xiT = np.empty((B, BL), bf)
    xjT = np.empty((B, BL), bf)
    for ci in range(N_CORES):
        sl = slice(ci * BL, (ci + 1) * BL)
        xiT[sl] = np.ascontiguousarray(x[tar_i[sl]].astype(bf).T)
        xjT[sl] = np.ascontiguousarray(x[tar_j[sl]].astype(bf).T)
    big = np.concatenate(
        [
            np.ascontiguousarray(ni).view(np.int16),
            np.ascontiguousarray(nj).view(np.int16),
            b2col.view(np.int16),
            xiT.view(np.int16),
            xjT.view(np.int16),
            np.tile(w1cat, (N_CORES, 1)).view(np.int16),
            xnj.view(np.int16),
        ],
        axis=1,
    )
    lay, listsw, bigw = _big_layout(si, sj)
    assert big.shape == (B, bigw), (big.shape, bigw)

    sml = np.concatenate([b1f, np.ones(BL, np.float32)]).reshape(1, DH + BL).astype(bf)

    in_maps = []
    for ci in range(N_CORES):
        sl = slice(ci * BL, (ci + 1) * BL)
        in_maps.append({
            "big": np.ascontiguousarray(big[sl]),
            "sml": sml,
        })
    return in_maps, si, sj, asg


def kernel(x, edge_index, tar_ei, W1, b1, W2, b2):
    from concourse.bass_utils import run_bass_kernel_spmd

    in_maps, si, sj, asg = _prepare(x, edge_index, tar_ei, W1, b1, W2, b2)

    key = (si, sj)
    if key not in _compiled_cache:
        _compiled_cache[key] = _build_bass(si, sj)
    nc = _compiled_cache[key]

    res = run_bass_kernel_spmd(nc, in_maps, list(range(N_CORES)))
    slots = np.concatenate(
        [res.results[ci]["out"].reshape(BL) for ci in range(N_CORES)]
    ).astype(np.float32)
    out = np.empty(B, np.float32)
    out[asg] = slots
    return out
